# revision 95
# baseline (speedup 1.0000x reference)
"""Trainium2 Bass kernel for the soft-decision-tree ensemble classifier (V5).

Restructure vs V3b: the deepest tree level is factored out of the log-domain
path matmul.  For parent node j (level-5 node 31+j), its two leaves satisfy
  lp[2j]   = exp(cm_j) * r5_j,      r5 = 1/(1+e^{z5})
  lp[2j+1] = exp(cm_j) * (1-r5_j)
with cm the level-0..4 path log-prob.  Folding into the output matmul with
V_A = V_odd, V_B = V_even - V_odd gives  out += V_A^T ep + V_B^T (ep*r5).
This halves the ScalarE exp/ln work (only 31 of 63 nodes need softplus), and
halves the A-matmul contraction (32-wide blocks, 4 trees per 128-tile).
The leaf-distribution softmax is computed on the host and DMA'd as V_A/V_B.
r5 comes from reciprocal_approx_fast (plain DVE reciprocal is ~7x slower);
the t1=te+1 add uses gpsimd tensor_tensor with a broadcast 1 (gpsimd
tensor_scalar is ~7x slower).  The deep (level-5) half of stage 1 runs as
fp8e4 DoubleRow matmuls with power-of-2 scales (w x512, x x32, descale 2^-14
in the DVE evac): z5 only feeds the sigmoid leaf split, so the fp8 noise
adds ~6e-3 scale-relative output error vs the 2e-2 budget; the shallow half
(which feeds exp-of-sums) stays fp16.  fp8 for the A/V matmul moving
operands was measured at 7-9e-2 error (leaf-prob sums are dominated by a
few leaves, errors don't average out) and rejected.

Sharding: 2-way trees x 4-way batch.  Per core: 32 trees in 8 groups of 4,
batch 1024 in 2 blocks of 512.  Pipeline: per pair of groups, head (stage-1
matmuls + bias evacs + exp/softplus) runs 1-2 pairs ahead of tail_a (A-
matmul + exp) and tail_b (epr + output matmul), chosen so each cross-engine
product is issued in the queue position where its consumer needs it.  PSUM:
pzs x2 + pzd x2 + pp x2 + out x2 = 8 banks exactly.  Warm-up matmuls on a
memset tile bridge the DMA head so the PE HAM clock-gate (1.2->2.4 GHz)
is released before real matmuls start; the final pair runs at half width
to shorten the drain.  DMA issue is spread over gpsimd/scalar/sync queues
(a dma_start costs ~0.9us of issue time on its queue) with piece order
matched to consumption order.
"""

import numpy as np

TREE_DEPTH = 6
T, N, D, C = 64, 63, 512, 100
L = 2**TREE_DEPTH
TG = 2
BG = 4
TL = T // TG               # 32 trees per core
NG = 8                     # tree groups of 4 per core
B = 4096
NCORES = 8
BSL = B // BG              # 1024 batch rows per core
VB = 512

N_WARM = 48

_NC_CACHE = {}


def _parent_paths():
    """For each level-0..4 parent path j: the shallow nodes visited and the
    direction bit at each, checking the level-5 node is 31+j."""
    paths = []
    for j in range(32):
        node = 0
        steps = []
        for k in range(5):
            bit = (j >> (4 - k)) & 1
            steps.append((node, bit))
            node = 2 * node + 1 + bit
        assert node == 31 + j
        paths.append(steps)
    return paths


def _pack_amat():
    """[128, 256]: [:, :128] = A'dir, [:, 128:] = A'path, block-diagonal
    over 4 trees of (32 shallow nodes x 32 parents).  fp8 for DoubleRow
    (entries are 0/±1, exact)."""
    import ml_dtypes
    adir = np.zeros((32, 32), np.float32)
    apath = np.zeros((32, 32), np.float32)
    for j, steps in enumerate(_parent_paths()):
        for node, bit in steps:
            if bit:
                adir[node, j] += 1.0
            apath[node, j] -= 1.0
    amat = np.zeros((128, 256), np.float32)
    for r in range(4):
        sl = slice(32 * r, 32 * r + 32)
        amat[sl, sl] = adir
        amat[sl, 128 + 32 * r:128 + 32 * r + 32] = apath
    return amat.astype(np.float16)


_AMAT = _pack_amat()
_EP_SCALE = 128.0
_LN_EP_SCALE = float(np.log(_EP_SCALE))


def _build_bass():
    import concourse.bacc as bacc
    import concourse.mybir as mybir
    import concourse.tile as tile
    from concourse.hw_specs import get_activation_tables

    dt = mybir.dt
    f32 = dt.float32
    f32r = dt.float32r
    fp16 = dt.float16
    fp8 = dt.float8e4
    AF = mybir.ActivationFunctionType
    ALU = mybir.AluOpType
    DR = mybir.MatmulPerfMode.DoubleRow

    nc = bacc.Bacc("TRN2", target_bir_lowering=False, debug=False,
                   num_devices=NCORES)

    table_id = next(i for i, (_, funcs) in
                    enumerate(get_activation_tables("gen3").items())
                    if AF.Exp in funcs and AF.Ln in funcs)
    nc.scalar.add_instruction(mybir.InstLoadActFuncSet(
        name=f"I-{nc.next_id()}", ins=[], outs=[], act_func_set_id=table_id))

    # ---- DRAM tensors ------------------------------------------------
    xt = nc.dram_tensor("xt", [128, 4096], fp16, kind="ExternalInput").ap()
    wt = nc.dram_tensor("wt", [4096, 128], fp16, kind="ExternalInput").ap()
    xt8 = nc.dram_tensor("xt8", [128, 4096], fp8,
                         kind="ExternalInput").ap()
    wt8 = nc.dram_tensor("wt8", [2048, 256], fp8,
                         kind="ExternalInput").ap()
    consts = nc.dram_tensor("consts", [128, 32], f32r,
                            kind="ExternalInput").ap()
    amat = nc.dram_tensor("amat", [128, 256], fp16,
                          kind="ExternalInput").ap()
    vmat = nc.dram_tensor("vmat", [128, 2048], fp16,
                          kind="ExternalInput").ap()
    outs = {}
    for v in range(2):
        for h in ("A", "B"):
            nm = f"o{h}{v}"
            outs[(v, h)] = nc.dram_tensor(nm, [100, VB], f32,
                                          kind="ExternalOutput").ap()

    with tile.TileContext(nc) as tc:
        with (
            tc.tile_pool(name="big", bufs=1) as bigp,
            tc.tile_pool(name="const", bufs=1) as constp,
            tc.tile_pool(name="ta", bufs=3) as tap,
            tc.tile_pool(name="work", bufs=3) as work,
            tc.tile_pool(name="pzs", bufs=2, space="PSUM") as pzsp,
            tc.tile_pool(name="pzd", bufs=2, space="PSUM") as pzdp,
            tc.tile_pool(name="pp", bufs=2, space="PSUM") as ppp,
            tc.tile_pool(name="po", bufs=2, space="PSUM") as pop,
        ):
            wt_t = bigp.tile([128, 4096], fp16, tag="wt")
            xt_t = bigp.tile([128, 4096], fp16, tag="xt")
            wt8_t = bigp.tile([128, 4096], fp8, tag="wt8")
            xt8_t = bigp.tile([128, 4096], fp8, tag="xt8")
            vm_t = bigp.tile([128, 2048], fp16, tag="vm")
            consts_t = constp.tile([128, 32], f32r, tag="consts")
            amat_t = constp.tile([128, 256], fp16, tag="amat")
            warm_t = constp.tile([128, 128], fp16, tag="warm")

            # ---- warm-up + DMA plan ----------------------------------
            # warm-up matmuls bridge the DMA fill so the PE HAM clock
            # gate is released before real matmuls start
            nc.gpsimd.memset(warm_t[:], 0.0)
            warm_ps = pop.tile([128, VB], f32, tag="ops", name="warm_ps")
            for _ in range(N_WARM):
                nc.tensor.matmul(warm_ps[:, 0:128], lhsT=warm_t[:],
                                 rhs=warm_t[:], start=True, stop=True)

            # dma_start issue costs ~0.9us on the issuing engine's queue:
            # scalar gets only early pieces (its ACTs start ~14us), gpsimd
            # a few (its tensor_tensor work starts ~16us), sync the bulk.
            def wt_piece(eng, j, g0, g1):
                eng.dma_start(
                    out=wt_t[:, j * 1024 + g0 * 128:j * 1024 + g1 * 128]
                    .rearrange("p (g c) -> p g c", c=128),
                    in_=wt[j * 1024 + g0 * 128:j * 1024 + g1 * 128, :]
                    .rearrange("(g p) c -> p g c", p=128))

            def wt8_piece(eng, g0, g1):
                eng.dma_start(
                    out=wt8_t[:, g0 * 512:g1 * 512]
                    .rearrange("p (g k c) -> p g k c", k=2, c=256),
                    in_=wt8[g0 * 256:g1 * 256, :]
                    .rearrange("(g k p) c -> p g k c", p=128, k=2))

            def xt_piece(eng, j, v):
                c0 = j * 1024 + v * VB
                eng.dma_start(out=xt_t[:, c0:c0 + VB], in_=xt[:, c0:c0 + VB])

            def xt8_piece(eng, v, k):
                c0 = v * 2048 + k * 1024
                eng.dma_start(out=xt8_t[:, c0:c0 + 1024],
                              in_=xt8[:, c0:c0 + 1024])

            nc.gpsimd.dma_start(out=consts_t[:], in_=consts[:])
            xt_piece(nc.gpsimd, 0, 0)
            wt_piece(nc.scalar, 0, 0, 2)
            xt_piece(nc.sync, 1, 0)
            wt_piece(nc.sync, 1, 0, 2)
            xt_piece(nc.gpsimd, 2, 0)
            wt_piece(nc.scalar, 2, 0, 2)
            xt_piece(nc.sync, 3, 0)
            wt_piece(nc.sync, 3, 0, 2)
            xt8_piece(nc.gpsimd, 0, 0)
            nc.scalar.dma_start(out=amat_t[:], in_=amat[:])
            xt8_piece(nc.sync, 0, 1)
            wt8_piece(nc.scalar, 0, 2)
            nc.scalar.dma_start(out=vm_t[:, 0:512], in_=vmat[:, 0:512])
            nc.scalar.dma_start(out=vm_t[:, 512:1024], in_=vmat[:, 512:1024])
            # bulk on sync (idle engine), medium pieces in need-order
            wt_piece(nc.sync, 0, 2, 5)
            wt_piece(nc.sync, 1, 2, 5)
            wt8_piece(nc.sync, 2, 5)
            wt_piece(nc.sync, 2, 2, 5)
            wt_piece(nc.sync, 3, 2, 5)
            wt_piece(nc.sync, 0, 5, 8)
            wt_piece(nc.sync, 1, 5, 8)
            wt8_piece(nc.sync, 5, 8)
            wt_piece(nc.sync, 2, 5, 8)
            wt_piece(nc.sync, 3, 5, 8)
            nc.sync.dma_start(out=vm_t[:, 1024:2048], in_=vmat[:, 1024:2048])
            xt_piece(nc.sync, 0, 1)
            xt_piece(nc.sync, 1, 1)
            xt8_piece(nc.sync, 1, 0)
            xt_piece(nc.sync, 2, 1)
            xt_piece(nc.sync, 3, 1)
            xt8_piece(nc.sync, 1, 1)

            adir_ap = amat_t[:, 0:128]
            apath_ap = amat_t[:, 128:256]

            def bias_ap(c):
                return consts_t[:, c:c + 1].bitcast(f32)

            ones_bc = (consts_t[:, 16:17].bitcast(f32)
                       .broadcast_to([128, 1024]))
            unscale_ap = consts_t[:, 17:18].bitcast(f32)

            # per-(v,half) output accumulators; created lazily
            out_ps = {}
            osb_tiles = {}

            # ---- pipeline --------------------------------------------
            # unit = (v, g).  pairs of consecutive groups share ACT batches.
            state = {}   # pair -> dict of tiles

            def head_mm(pair):
                v, ga = pair
                # both groups' shallow matmuls first: they feed the critical
                # softplus chain, and a deep matmul stalled on the (later)
                # fp8 DMA pieces must not FIFO-block them
                pzss, pzds = [], []
                for h, g in enumerate((ga, ga + 1)):
                    pzs = pzsp.tile([128, VB], f32, tag="pzs")
                    pzss.append(pzs)
                    for j in range(4):
                        nc.tensor.matmul(
                            pzs[:],
                            lhsT=wt_t[:, j * 1024 + g * 128:
                                      j * 1024 + (g + 1) * 128],
                            rhs=xt_t[:, j * 1024 + v * VB:
                                     j * 1024 + (v + 1) * VB],
                            start=(j == 0), stop=(j == 3),
                        )
                for h, g in enumerate((ga, ga + 1)):
                    pzd = pzdp.tile([128, VB], f32, tag="pzd")
                    pzds.append(pzd)
                    for kk in range(2):
                        nc.tensor.matmul(
                            pzd[:],
                            lhsT=wt8_t[:, g * 512 + kk * 256:
                                       g * 512 + (kk + 1) * 256]
                            .rearrange("p (two m) -> p two m", two=2),
                            rhs=xt8_t[:, v * 2048 + kk * 1024:
                                      v * 2048 + (kk + 1) * 1024]
                            .rearrange("p (two n) -> p two n", two=2),
                            start=(kk == 0), stop=(kk == 1),
                            perf_mode=DR,
                        )
                state[pair] = dict(pzss=pzss, pzds=pzds)

            def head_act(pair, narrow=False, dp_first=False):
                v, ga = pair
                st = state[pair]
                pzss, pzds = st.pop("pzss"), st.pop("pzds")
                # ta2 layout: [sh_g | sh_g1 | dp_g | dp_g1]
                ta2 = tap.tile([128, 2048], fp16, tag="ta",
                               name=f"ta_{v}_{ga}")
                # shallow evacs first so sp's inputs are ready earliest
                for h, g in enumerate((ga, ga + 1)):
                    nc.vector.tensor_scalar_add(
                        out=ta2[:, h * VB:(h + 1) * VB],
                        in0=pzss[h][:], scalar1=bias_ap(2 * g))
                for h, g in enumerate((ga, ga + 1)):
                    nc.vector.tensor_scalar(
                        out=ta2[:, 1024 + h * VB:1024 + (h + 1) * VB],
                        in0=pzds[h][:],
                        scalar1=2.0 ** -14, scalar2=bias_ap(2 * g + 1),
                        op0=ALU.mult, op1=ALU.add)
                te2 = work.tile([128, 2048], f32, tag="te",
                                name=f"te_{v}_{ga}")
                sp2 = work.tile([128, 1024], fp16, tag="sp",
                                name=f"sp_{v}_{ga}")
                hw = VB if narrow else 1024

                def sh_acts():
                    for o in range(0, 1024, hw):
                        nc.scalar.activation(te2[:, o:o + hw],
                                             ta2[:, o:o + hw], AF.Exp)
                        nc.scalar.activation(sp2[:, o:o + hw],
                                             te2[:, o:o + hw],
                                             AF.Ln, bias=1.0)

                def dp_acts():
                    for o in range(0, 1024, hw):
                        nc.scalar.activation(te2[:, 1024 + o:1024 + o + hw],
                                             ta2[:, 1024 + o:1024 + o + hw],
                                             AF.Exp)

                # near the drain, the deep exp (which feeds the t1->r5->epr
                # chain, the drain-critical path) goes first
                if dp_first:
                    dp_acts()
                    sh_acts()
                else:
                    sh_acts()
                    dp_acts()
                state[pair].update(ta2=ta2, sp2=sp2, te2=te2, hw=hw)
                if narrow:
                    # last pair: pull the r5 chain out of the drain path
                    t1 = work.tile([128, 1024], f32, tag="t1",
                                   name=f"t1_{v}_{ga}")
                    r5 = work.tile([128, 1024], f32, tag="r5",
                                   name=f"r5_{v}_{ga}")
                    for o in range(0, 1024, VB):
                        # t1 on DVE here: a gpsimd t1 would head-of-line
                        # block the previous pair's epr at the drain
                        nc.vector.tensor_scalar_add(
                            out=t1[:, o:o + VB],
                            in0=te2[:, 1024 + o:1024 + o + VB],
                            scalar1=1.0)
                        nc.vector.reciprocal_approx_fast(
                            out=r5[:, o:o + VB], in_=t1[:, o:o + VB])
                    state[pair]["t1"] = t1
                    state[pair]["r5"] = r5

            def tail_a(pair, cw=VB):
                v, ga = pair
                st = state[pair]
                ta2, sp2, te2 = st["ta2"], st["sp2"], st["te2"]

                pps = [ppp.tile([128, VB], f32, tag="pp", name=f"pp{h}")
                       for h in range(2)]
                ep = work.tile([128, 1024], fp16, tag="ep",
                               name=f"ep_{v}_{ga}")
                for off in range(0, 1024, cw):
                    h, inner = off // VB, off % VB
                    psl = slice(inner, inner + cw)
                    esl = slice(off, off + cw)
                    nc.tensor.matmul(pps[h][:, psl], lhsT=adir_ap,
                                     rhs=ta2[:, esl],
                                     start=True, stop=False)
                    nc.tensor.matmul(pps[h][:, psl], lhsT=apath_ap,
                                     rhs=sp2[:, esl],
                                     start=False, stop=True)
                    nc.scalar.activation(ep[:, esl], pps[h][:, psl], AF.Exp)
                if "r5" not in st:
                    t1 = work.tile([128, 1024], f32, tag="t1",
                                   name=f"t1_{v}_{ga}")
                    nc.gpsimd.tensor_tensor(out=t1[:], in0=te2[:, 1024:2048],
                                            in1=ones_bc, op=ALU.add)
                    r5 = work.tile([128, 1024], f32, tag="r5",
                                   name=f"r5_{v}_{ga}")
                    nc.vector.reciprocal_approx_fast(out=r5[:], in_=t1[:])
                    st["r5"] = r5
                # epr issued here (a full pair-cycle before its s3 matmuls)
                r5 = st["r5"]
                epr = work.tile([128, 1024], fp16, tag="epr",
                                name=f"epr_{v}_{ga}")
                for i, off in enumerate(range(0, 1024, cw)):
                    esl = slice(off, off + cw)
                    eng = nc.vector if (cw < VB and i % 2 == 1) else nc.gpsimd
                    eng.tensor_tensor(out=epr[:, esl], in0=ep[:, esl],
                                      in1=r5[:, esl], op=ALU.mult)
                st["ep"] = ep
                st["epr"] = epr

            def tail_b(pair, cw=VB, last=False):
                v, ga = pair
                st = state.pop(pair)
                ep, epr = st["ep"], st["epr"]
                half = "A" if ga < 4 else "B"
                if (v, half) not in out_ps:
                    out_ps[(v, half)] = pop.tile([128, VB], f32, tag="ops",
                                                 name=f"ops_{v}{half}")
                ops = out_ps[(v, half)]
                offs = list(range(0, 1024, cw))
                for i, off in enumerate(offs):
                    g, inner = ga + off // VB, off % VB
                    esl = slice(off, off + cw)
                    bsl = slice(inner, inner + cw)
                    nc.tensor.matmul(ops[:, bsl],
                                     lhsT=vm_t[:, g * 256:g * 256 + 128],
                                     rhs=ep[:, esl],
                                     start=(g % 4 == 0 and inner == 0),
                                     stop=False)
                    nc.tensor.matmul(ops[:, bsl],
                                     lhsT=vm_t[:, g * 256 + 128:g * 256 + 256],
                                     rhs=epr[:, esl],
                                     start=False,
                                     stop=(g % 4 == 3 and i == len(offs) - 1))
                if ga + 1 in (3, 7):
                    osb = work.tile([128, VB], f32, tag=f"osb{half}",
                                    name=f"osb_{v}{half}")
                    nc.vector.tensor_copy(out=osb[:], in_=ops[:])
                    eng = nc.gpsimd if half == "A" else nc.sync
                    eng.dma_start(out=outs[(v, half)][:], in_=osb[0:100, :])
                    del out_ps[(v, half)]

            pairs = [(v, ga) for v in range(2) for ga in (0, 2, 4, 6)]
            for i, p in enumerate(pairs):
                head_mm(p)
                if i < 3:
                    # extra warm-up bursts: keep the PE HAM clock-gate open
                    # through the DMA-sparse fill phase
                    for _ in range(8):
                        nc.tensor.matmul(warm_ps[:, 0:128], lhsT=warm_t[:],
                                         rhs=warm_t[:], start=True, stop=True)
                if i == len(pairs) - 1:
                    # drain: the previous pair's ep goes ahead of the last
                    # head's te/sp in the ACT queue (nothing left for ACT
                    # to starve on at this point)
                    tail_a(pairs[i - 1], cw=256)
                    head_act(p, narrow=True, dp_first=True)
                else:
                    head_act(p, narrow=False, dp_first=(i >= len(pairs) - 3))
                    if i >= 1:
                        tail_a(pairs[i - 1], cw=VB)
                if i >= 2:
                    tail_b(pairs[i - 2],
                           cw=256 if i == len(pairs) - 1 else VB)
            tail_a(pairs[-1], cw=256)
            tail_b(pairs[-2], cw=256)
            tail_b(pairs[-1], cw=256, last=True)

    nc.finalize()
    return nc


def _get_nc():
    if "nc" not in _NC_CACHE:
        _NC_CACHE["nc"] = _build_bass()
    return _NC_CACHE["nc"]


def _prep_inputs(x, split_weights, split_bias, leaf_logits, tree_weights):
    x = np.asarray(x, np.float32)
    split_weights = np.asarray(split_weights, np.float32)
    split_bias = np.asarray(split_bias, np.float32)
    leaf_logits = np.asarray(leaf_logits, np.float32)
    tree_weights = np.asarray(tree_weights, np.float32)

    w_soft = np.exp(tree_weights - tree_weights.max())
    w_soft = w_soft / w_soft.sum()

    # leaf distributions scaled by 2*w_t (host softmax)
    ll = leaf_logits - leaf_logits.max(axis=-1, keepdims=True)
    ev = np.exp(ll)
    dist = ev / ev.sum(axis=-1, keepdims=True)          # [T, L, C]
    vt = 2.0 * w_soft[:, None, None] * dist             # [T, L, C]

    import ml_dtypes

    in_maps = []
    for tg in range(TG):
        trees = np.arange(tg * TL, (tg + 1) * TL)
        # shallow weights fp16: wt_np[j, g, p, m]; deep weights fp8
        # (scaled x512) packed for DoubleRow: wt8_np[g, k, p, tj, m]
        wt_np = np.zeros((4, NG, 128, 128), np.float32)
        wt8_np = np.zeros((NG, 2, 128, 2, 128), np.float32)
        bias_np = np.zeros((128, 32), np.float32)
        vm_np = np.zeros((128, NG, 2, 128), np.float32)
        for tl, t in enumerate(trees):
            g, r = tl // 4, tl % 4
            rows = slice(32 * r, 32 * r + 32)
            W = split_weights[t]                        # [N, D]
            bs = split_bias[t]                          # [N]
            # shallow nodes 0..30 (+pad), deep nodes 31..62
            wsh = np.zeros((32, D), np.float32)
            wsh[:31] = W[0:31]
            wdp = W[31:63]                              # [32, D]
            for j in range(4):
                dsl = slice(j * 128, (j + 1) * 128)
                wt_np[j, g, :, 32 * r:32 * r + 32] = wsh[:, dsl].T
            wq = np.clip(wdp * 512.0, -224, 224).T      # [D, 32]
            wt8_np[g, :, :, :, 32 * r:32 * r + 32] = (
                wq.reshape(2, 2, 128, 32).transpose(0, 2, 1, 3))
            bias_np[rows, 2 * g] = np.concatenate([bs[0:31], [0.0]])
            bias_np[rows, 2 * g + 1] = bs[31:63]
            # V_A = V_odd, V_B = V_even - V_odd  (parents j = 0..31)
            v_even = vt[t, 0::2, :]                     # [32, C]
            v_odd = vt[t, 1::2, :]
            vm_np[rows, g, 0, :C] = v_odd
            vm_np[rows, g, 1, :C] = v_even - v_odd
        wt16 = np.ascontiguousarray(
            wt_np.reshape(4096, 128).astype(np.float16))
        wt8a = np.ascontiguousarray(
            wt8_np.reshape(2048, 256).astype(ml_dtypes.float8_e4m3))
        vm16 = np.ascontiguousarray(
            vm_np.reshape(128, 2048).astype(np.float16))
        bias_np[:, 16] = 1.0
        shared = dict(wt=wt16, wt8=wt8a, consts=bias_np.copy(), amat=_AMAT,
                      vmat=vm16)
        for bg in range(BG):
            xs = x[bg * BSL:(bg + 1) * BSL, :]          # [1024, 512]
            xt16 = np.ascontiguousarray(
                xs.T.reshape(4, 128, BSL).transpose(1, 0, 2)
                .reshape(128, 4096).astype(np.float16))
            xq = np.clip(xs.T * 32.0, -224, 224)        # [512, 1024]
            xt8a = np.ascontiguousarray(
                xq.reshape(2, 2, 128, 2, VB).transpose(2, 3, 0, 1, 4)
                .reshape(128, 4096).astype(ml_dtypes.float8_e4m3))
            in_maps.append(dict(xt=xt16, xt8=xt8a, **shared))
    return in_maps


def kernel(x, split_weights, split_bias, leaf_logits, tree_weights):
    from concourse.bass_utils import run_bass_kernel_spmd

    in_maps = _prep_inputs(x, split_weights, split_bias, leaf_logits,
                           tree_weights)
    nc = _get_nc()
    res = run_bass_kernel_spmd(nc, in_maps, core_ids=list(range(NCORES)))
    out = np.zeros((B, C), np.float32)
    for tg in range(TG):
        for bg in range(BG):
            r = res.results[tg * BG + bg]
            for v in range(2):
                part = (r[f"oA{v}"] + r[f"oB{v}"]).T      # [512, 100]
                rows = slice(bg * BSL + v * VB, bg * BSL + (v + 1) * VB)
                out[rows] += part
    return np.ascontiguousarray(out)
